# revision 43
# baseline (speedup 1.0000x reference)
"""Trainium2 Bass kernel for nn_MoELayer (moe_routing).

Reference computation (B=8192 tokens, d=1024, E=8 experts, top-k=2):
    gating  = softmax(x @ gate_w + gate_b)                    # [B, E]
    mask    = top-2 one-hot scatter of gating                 # [B, E]
    blockm  = mask.reshape(B//d, d, E).max(axis=1)            # per 1024-row block
    out     = sum_e gating[:, e] * blockm[block(b), e] * (x @ W[:, e*d:(e+1)*d])

Key structural facts exploited here:
  * The combine uses the FULL softmax weights over all experts; the top-2
    mask enters only through the per-1024-row-block max. So the compute is
    dense: out = sum_e (gating*blockmask) .* (x @ W_e).
  * Both the gating and the block mask for a 1024-row block depend only on
    that block's rows.

Sharding: data-parallel over the 8 row blocks of 1024 tokens (one per
NeuronCore). Each core computes its whole output block locally; there is
no cross-core communication. W is streamed (replicated) to every core.

Precision: experts 5 and 6 are computed in fp8 e4m3 with
perf_mode=DoubleRow (two k-tiles contracted per matmul -> ~2x the bf16
MAC rate; a K=256,N=512 DoubleRow matmul measures ~same ns as a K=128
bf16 one). Host quantizes x*4 and W_e*128 to e4m3; the 1/512 dequant is
folded into the per-expert block-mask row, so the combine path is
unchanged. Measured end-to-end rel err 1.89e-2 vs the 2e-2 gate
(bf16-only is 0.33e-2; each fp8 expert adds ~1.33e-2 in quadrature -
exactly two experts fit the budget). This cut HW time 241.3us -> 214.2us.

Schedule (PE-bound: 8x 1024^3 bf16 matmuls ~ 218us at 2.4 GHz):
  * All inputs are converted to bf16 on the host: input DMA halves
    (18.8 MB/core) and no on-device casts gate the matmul stream.
  * x^T tiles stream on the Scalar engine's HW DMA queue; the W stream
    and outputs use the Sync queue — two queues so the ~0.7us/instr
    descriptor-generation cost doesn't serialize at startup.
  * Expert 0 is computed UNSCALED in two k-half passes (starts as soon
    as the first k-tiles arrive, no gating dependency); expert 1 likewise.
    Their gating scales are folded in during experts 3-6.
  * Gating logits ride the expert-0 stream as N=8 matmuls reusing the
    already-loaded x-tile stationary (k-accumulated into one PSUM bank),
    + K=1 bias matmuls; softmax/top-2/block-mask run under expert 1.
  * Experts 2..6: bf16 matmuls, fp32 PSUM k-accumulation; ACT applies the
    per-token gating scale PSUM->SBUF; DVE accumulates into acc.
  * Expert 7 runs half-major (all m-tiles of columns 0-511, then 512-1023)
    writing bf16 output tiles; each half's output DMA overlaps the rest.
  * Output is bf16 (host upcasts to fp32): halves the output DMA.
  * A short PE warm-up keeps the HAM clock-gate at 2.4 GHz from the start.
"""

import numpy as np

P = 128          # partitions
D = 1024         # d_model
E = 8            # experts
TOK = 1024       # tokens per core (row block)
KT = D // P      # contraction tiles
KH = KT // 2     # k-half for expert 0's two passes
MT = TOK // P    # token tiles
NH = 512         # psum half-width (one fp32 bank)
N_CORES = 8
WARMUP_MMS = 22  # N=128 PE warm-up matmuls issued while the first DMAs land
                 # (22 is tuned: pass A is DMA-arrival-paced, and extra
                 # warm-up MMs sit in FRONT of it on the in-order PE queue -
                 # 36 was measured +1.5us end-to-end)

# Experts computed in fp8 e4m3 with DoubleRow (2 fp8 weights/PE cell -> a
# K=256 matmul per instruction, ~1.8x the bf16 MAC rate). The e4m3
# quantization noise of one full expert adds ~1.3% rel error (measured
# end-to-end: bf16-only 0.33%, per-expert fp8 variance 4*0.44%^2); the
# 2e-2 gate leaves room for 2 experts (~1.9%). The dequant scale
# 1/(SXQ*SWQ) rides the per-expert block-mask row bm01, so the combine
# path is unchanged. Experts 0 (xw-packed) and 7 (output tail) stay bf16.
FP8_EXPERTS = (5, 6)
KP = KT // 2     # DoubleRow k-tile pairs per expert
SXQ = 4.0        # x -> e4m3 scale (|4x| <= ~22 << 240)
SWQ = 128.0      # W -> e4m3 scale (lifts W*0.01 out of the subnormal range)
NF8 = len(FP8_EXPERTS)


def _build_nc():
    import concourse.bacc as bacc
    import concourse.mybir as mybir
    import concourse.tile as tile

    f32 = mybir.dt.float32
    bf16 = mybir.dt.bfloat16
    f8 = mybir.dt.float8e4
    PM = mybir.MatmulPerfMode.DoubleRow
    AX = mybir.AxisListType
    OP = mybir.AluOpType
    AF = mybir.ActivationFunctionType

    # Bacc (not raw Bass): its compile() pass splits excess per-instruction
    # semaphore waits into standalone event-semaphore instructions and moves
    # matmul waits onto LDWEIGHTS — required for walrus codegen's per-
    # instruction sync-wait limits.
    assert 0 not in FP8_EXPERTS and (E - 1) not in FP8_EXPERTS
    nc = bacc.Bacc(None, target_bir_lowering=False, debug=False)
    # xw0: host-packed [xT | W_expert0] row blocks - one DMA per k-tile
    # delivers BOTH of the first matmuls' operands with one issue slot and
    # one completion event (the startup trickle is issue-rate + BW bound).
    xw_d = nc.dram_tensor("xw0", [D, 2 * D], bf16, kind="ExternalInput")
    w_d = nc.dram_tensor("w", [D, E * D], bf16, kind="ExternalInput")
    x8_d = nc.dram_tensor("x8", [D, TOK], f8, kind="ExternalInput")
    w8_d = nc.dram_tensor("w8", [D, NF8 * D], f8, kind="ExternalInput")
    gw_d = nc.dram_tensor("gate_w", [D, E], bf16, kind="ExternalInput")
    gb_d = nc.dram_tensor("gate_b", [1, MT * E], bf16, kind="ExternalInput")
    out_d = nc.dram_tensor("out", [TOK, D], bf16, kind="ExternalOutput")

    xw_r = xw_d.rearrange("(k p) f -> k p f", p=P)
    w_r2 = w_d.rearrange("(k p) (e f) -> p e k f", p=P, f=D)
    x8_r = x8_d.rearrange("(k p) t -> p k t", p=P)
    w8_r = w8_d.rearrange("(k p) (i f) -> p i k f", p=P, f=D)
    gw_r = gw_d.rearrange("(k p) e -> p k e", p=P)
    out_r = out_d.rearrange("(m p) f -> m p f", p=P)

    with tile.TileContext(nc) as tc:
        with (
            tc.tile_pool(name="persist", bufs=1) as persist,
            tc.tile_pool(name="gstat", bufs=2) as p_gs,
            tc.tile_pool(name="wb", bufs=KT) as p_wb,
            tc.tile_pool(name="wbig", bufs=2) as p_wbig,
            tc.tile_pool(name="w8p", bufs=2) as p_w8,
            tc.tile_pool(name="ps_lg", bufs=1, space="PSUM") as ps_lg,
            tc.tile_pool(name="ps_mm", bufs=7, space="PSUM") as ps_mm,
        ):
            # -- input DMAs first: nothing depends on compute, so they are
            # issued before any front-matter. ALL PE-consumed inputs ride
            # the ONE sync HW queue: a matmul whose lhsT and rhs complete on
            # different DMA queues needs two DMA-queue waits, which the
            # LDWEIGHTS wait encoding can't take. x/w0 k-tiles interleaved
            # so the first matmul's operands land earliest; gate tensors
            # follow (their matmuls run in pass B, ~15us later).
            # ALL PE-consumed inputs ride the ONE sync HW queue: splitting
            # x onto the scalar queue was measured 12us SLOWER - every
            # expert matmul then needs waits on two DMA queues, which Bacc
            # legalizes as standalone event-semaphore instructions that
            # throttle the in-order PE queue for the whole stream.
            xtb = []
            wbf0 = []
            for k in range(KT):
                xw = p_wb.tile([P, 2 * D], bf16, tag="xw", name=f"xw{k}")
                nc.sync.dma_start(xw[:], xw_r[k])
                xtb.append(xw[:, 0:D])
                wbf0.append(xw[:, D:2 * D])
            gw_in = persist.tile([P, KT, E], bf16, tag="gw_in")
            nc.sync.dma_start(gw_in[:], gw_r[:])
            # gate_b host-tiled to [1, MT*E] so ONE K=1 matmul adds the
            # bias to every m-tile's logit region at once.
            gb_in = persist.tile([1, MT * E], bf16, tag="gb_in")
            nc.sync.dma_start(gb_in[:], gb_d[:])

            # -- front matter (no DMA dependency).
            # warm-up operand memsets on GpSimd: its preamble ends ~2us
            # before the Vector engine's, so the PE warm-up (and the HAM
            # clock ramp) starts pre-window instead of at ~7us. (A DMA-fed
            # variant was measured WORSE: the scalar HW queue doesn't move
            # user data until ~13us, stalling the whole in-order PE queue.)
            wu_t = persist.tile([P, 2 * P], bf16, tag="wu_t")
            nc.gpsimd.memset(wu_t[:], 0.0)
            wu_lhs = wu_t[:, 0:P]
            wu_rhs = wu_t[:, P:2 * P]
            ones_col = persist.tile([P, 1], bf16, tag="ones_col")
            nc.vector.memset(ones_col[:], 1.0)
            exp_in = persist.tile([P, 1], f32, tag="exp_in")
            nc.vector.memset(exp_in[:], 1.0)
            ones_row_bf = persist.tile([1, P], bf16, tag="ones_row_bf")
            nc.vector.memset(ones_row_bf[:], 1.0)
            # per-expert dequant row: 1.0, except 1/(SXQ*SWQ) for fp8
            # experts (folded into the block-mask row bm01 below).
            dq_row = persist.tile([1, E], bf16, tag="dq_row")
            nc.vector.memset(dq_row[:], 1.0)
            for _e in FP8_EXPERTS:
                nc.vector.memset(dq_row[:, _e:_e + 1], 1.0 / (SXQ * SWQ))
            # Preload the exp activation-table set (~2.7us) during DMA wait.
            exp_dummy = persist.tile([1, 1], f32, tag="exp_dummy")
            nc.scalar.activation(exp_dummy[:], exp_in[:1, :], AF.Exp)

            # PE warm-up while the first input DMAs are in flight: a dense
            # block of short N=128 matmuls keeps the HAM activity monitor
            # busy (and the clock ramping) until the real stream's operands
            # land, without ever delaying a ready matmul by more than
            # ~110ns. Warm-up, pass-A fillers and pass-B gating logits are
            # strictly sequential, so they time-share ONE PSUM bank (the
            # ps_lg pool) and the matmul pool gets 7 banks - a 6-bank pool
            # wraps every 3rd (m,e) iteration and was measured stalling the
            # PE ~216ns every 10.8us waiting on the DVE drain.
            wu_ps = ps_lg.tile([P, P], f32, tag="lg", bufs=1)
            for i in range(WARMUP_MMS):
                nc.tensor.matmul(
                    wu_ps[:], wu_lhs, wu_rhs,
                    start=(i == 0), stop=(i == WARMUP_MMS - 1),
                )

            bmb = persist.tile([P, E], f32, tag="bmb")

            # acc: the fp32 output accumulator. acc0/acc1: experts 0/1
            # computed UNSCALED (bf16; same precision as the bf16 matmul
            # inputs already cost). Their gating scales are folded in
            # during experts 3-6, off the startup critical path.
            acc = []
            acc0 = []
            acc1 = []
            outb = []
            for m in range(MT):
                acc.append(persist.tile([P, D], f32, tag=f"acc{m}",
                                        name=f"acc{m}"))
                acc0.append(persist.tile([P, D], bf16, tag=f"acc0{m}",
                                         name=f"acc0{m}"))
                acc1.append(persist.tile([P, D], bf16, tag=f"acc1{m}",
                                         name=f"acc1{m}"))
                outb.append(persist.tile([P, D], bf16, tag=f"outb{m}",
                                         name=f"outb{m}"))

            # Dependency-free filler matmul: keeps the PE's HAM activity
            # monitor busy during arrival-gated stretches so the clock stays
            # at 2.4 GHz.
            def pe_filler(n=1):
                for _ in range(n):
                    nc.tensor.matmul(wu_ps[:], wu_lhs, wu_rhs,
                                     start=True, stop=True)

            # -- expert 0, pass A (k = 0..3), unscaled -> acc0.
            for m in range(MT):
                ps0 = ps_mm.tile([P, NH], f32, tag="psmm")
                ps1 = ps_mm.tile([P, NH], f32, tag="psmm")
                for k in range(KH):
                    lhs = xtb[k][:, m * P:(m + 1) * P]
                    nc.tensor.matmul(ps0[:], lhs, wbf0[k][:, 0:NH],
                                     start=(k == 0), stop=(k == KH - 1))
                    nc.tensor.matmul(ps1[:], lhs, wbf0[k][:, NH:D],
                                     start=(k == 0), stop=(k == KH - 1))
                    if m == 0 and k > 0:
                        # 8 fillers + 2 cold wides ~= the 1.64us packed-pair
                        # DMA arrival cadence, so m0's k-loop never idles
                        # (measured ~1us of recurring idle at 4 fillers).
                        # k==0 needs none: the warm-up block still runs when
                        # pair 0 lands.
                        pe_filler(8)
                nc.scalar.copy(acc0[m][:, 0:NH], ps0[:])
                nc.scalar.copy(acc0[m][:, NH:D], ps1[:])

            def load_w(e):
                # one dma_start for the whole expert ([P, KT, D] tile):
                # fewer sync-queue instructions and end-of-kernel events.
                wb = p_wbig.tile([P, KT, D], bf16, tag="wbig", name=f"wb{e}")
                nc.sync.dma_start(wb[:], w_r2[:, e])
                return [wb[:, k, :] for k in range(KT)]

            def load_w8(e):
                # fp8 expert weights: [P, k-tile, D] e4m3, half the bytes.
                i = FP8_EXPERTS.index(e)
                wb = p_w8.tile([P, KT, D], f8, tag="w8", name=f"w8_{e}")
                nc.sync.dma_start(wb[:], w8_r[:, i])
                return wb

            # Prefetch expert 1's weights ahead of pass B.
            wbf_cur = load_w(1)
            # fp8 x^T (stationary side of the DoubleRow matmuls): issued
            # after W1 so it cannot delay pass B; first consumed by the
            # earliest fp8 expert, many tens of us later.
            x8t = persist.tile([P, KT, TOK], f8, tag="x8t")
            nc.sync.dma_start(x8t[:], x8_r[:])

            # Gating logits accumulate in one PSUM bank: all 8 m-tiles side
            # by side ([P, MT*E]), each m-tile k-accumulated by the tiny
            # matmul chains riding pass B below. Allocated here (after the
            # last pe_filler) because the bank is time-shared with the
            # warm-up/filler tile.
            lg_all = ps_lg.tile([P, MT * E], f32, tag="lg", bufs=1)

            # -- expert 0, pass B (k = 4..7), accumulate into acc0 on DVE.
            # Each m's gating logits ride along as a contiguous chain of
            # N=8 matmuls over all k (reusing resident x-tiles) + the K=1
            # bias matmul. One chain per m, start->stop contiguous in the
            # lg bank (start=True's whole-bank has_written clear only
            # resets bits of finished chains, whose values just get read).
            for m in range(MT):
                ps0 = ps_mm.tile([P, NH], f32, tag="psmm")
                ps1 = ps_mm.tile([P, NH], f32, tag="psmm")
                for k in range(KH, KT):
                    lhs = xtb[k][:, m * P:(m + 1) * P]
                    nc.tensor.matmul(ps0[:], lhs, wbf0[k][:, 0:NH],
                                     start=(k == KH), stop=(k == KT - 1))
                    nc.tensor.matmul(ps1[:], lhs, wbf0[k][:, NH:D],
                                     start=(k == KH), stop=(k == KT - 1))
                for k in range(KT):
                    # single start=True on the very first gating matmul: it
                    # clears has_written for the whole bank; every other
                    # region's first write overwrites-then-sets, later k's
                    # accumulate.
                    nc.tensor.matmul(lg_all[:, m * E:(m + 1) * E],
                                     xtb[k][:, m * P:(m + 1) * P],
                                     gw_in[:, k, :],
                                     start=(m == 0 and k == 0), stop=False)
                nc.vector.tensor_tensor(acc0[m][:, 0:NH], acc0[m][:, 0:NH],
                                        ps0[:], op=OP.add)
                nc.vector.tensor_tensor(acc0[m][:, NH:D], acc0[m][:, NH:D],
                                        ps1[:], op=OP.add)
            # one K=1 matmul adds the (host-tiled) bias to ALL m-regions
            # and closes the whole logit accumulation.
            nc.tensor.matmul(lg_all[:], ones_row_bf[:], gb_in[:],
                             start=False, stop=True)

            # Gating part 2: ONE ACT Exp over the whole [P, MT*E] logit bank
            # (a single PSUM read, so the PE's later bank writes never
            # ping-pong with per-m reads), then the softmax / top-2-mask
            # chain on SBUF slices. The cross-m reductions collapse to one
            # strided tensor_reduce each.
            ex_all = persist.tile([P, MT * E], f32, tag="ex_all")
            nc.scalar.activation(ex_all[:], lg_all[:], AF.Exp)
            ssum_all = p_gs.tile([P, MT], f32, tag="ssum_all", bufs=1)
            nc.vector.tensor_reduce(
                ssum_all[:], ex_all[:].rearrange("p (m e) -> p m e", e=E),
                axis=AX.X, op=OP.add,
            )
            rcp_all = p_gs.tile([P, MT], f32, tag="rcp_all", bufs=1)
            nc.vector.reciprocal(rcp_all[:], ssum_all[:])
            m1_all = p_gs.tile([P, MT], f32, tag="m1_all", bufs=1)
            nc.vector.tensor_reduce(
                m1_all[:], ex_all[:].rearrange("p (m e) -> p m e", e=E),
                axis=AX.X, op=OP.max,
            )
            mask_all = persist.tile([P, MT * E], bf16, tag="mask_all")
            gfin = []
            gsc = [persist.tile([P, E], f32, tag=f"gsc{m}", name=f"gsc{m}")
                   for m in range(MT)]
            for m in range(MT):
                ex = ex_all[:, m * E:(m + 1) * E]
                # top-2 mask: v >= (max of values with the argmax removed)
                eqb = p_gs.tile([P, E], f32, tag="eqb")
                nc.vector.tensor_scalar(
                    eqb[:], ex, m1_all[:, m:m + 1], -1e30,
                    op0=OP.is_ge, op1=OP.mult
                )
                g2 = p_gs.tile([P, E], f32, tag="g2")
                nc.vector.tensor_tensor(g2[:], ex, eqb[:], op=OP.add)
                m2 = p_gs.tile([P, 1], f32, tag="m2")
                nc.vector.reduce_max(m2[:], g2[:], axis=AX.X)
                nc.vector.tensor_scalar(mask_all[:, m * E:(m + 1) * E],
                                        ex, m2[:], None, op0=OP.is_ge)
                gt = p_gs.tile([P, E], f32, tag=f"gt{m}", bufs=1)
                nc.vector.tensor_scalar_mul(gt[:], ex, rcp_all[:, m:m + 1])
                gfin.append(gt)

            # -- experts 1..6: acc (+)= g_e * (x @ W_e) via ONE fused DVE
            # op per half (scalar_tensor_tensor: (ps*g)+acc). Expert 0/1's
            # scaled contributions are merged in during experts 3-6. The
            # block-mask matmuls ride the PE queue inside expert 1's m-loop
            # (late enough that the softmax chain has finished - placing
            # them right after pass B would stall the in-order PE queue).
            for e in range(1, E - 1):
                wbf = wbf_cur
                cur_f8 = e in FP8_EXPERTS
                nxt = e + 1
                wbf_cur = load_w8(nxt) if nxt in FP8_EXPERTS else load_w(nxt)
                for m in range(MT):
                    ps0 = ps_mm.tile([P, NH], f32, tag="psmm")
                    ps1 = ps_mm.tile([P, NH], f32, tag="psmm")
                    if cur_f8:
                        # DoubleRow: each matmul contracts a PAIR of
                        # k-tiles ([128, 2, *] APs) at ~2x the MAC rate.
                        for j in range(KP):
                            lhs = x8t[:, 2 * j:2 * j + 2, m * P:(m + 1) * P]
                            nc.tensor.matmul(ps0[:], lhs,
                                             wbf[:, 2 * j:2 * j + 2, 0:NH],
                                             start=(j == 0),
                                             stop=(j == KP - 1), perf_mode=PM)
                            nc.tensor.matmul(ps1[:], lhs,
                                             wbf[:, 2 * j:2 * j + 2, NH:D],
                                             start=(j == 0),
                                             stop=(j == KP - 1), perf_mode=PM)
                    else:
                        for k in range(KT):
                            lhs = xtb[k][:, m * P:(m + 1) * P]
                            nc.tensor.matmul(ps0[:], lhs, wbf[k][:, 0:NH],
                                             start=(k == 0),
                                             stop=(k == KT - 1))
                            nc.tensor.matmul(ps1[:], lhs, wbf[k][:, NH:D],
                                             start=(k == 0),
                                             stop=(k == KT - 1))
                    for h, ps in ((0, ps0), (1, ps1)):
                        osl = acc[m][:, h * NH:(h + 1) * NH]
                        if e == 1:
                            # expert 1 is also computed unscaled (no gating
                            # dependency); merged with its gate later.
                            nc.scalar.copy(acc1[m][:, h * NH:(h + 1) * NH],
                                           ps[:])
                        elif e == 2:
                            nc.scalar.mul(osl, ps[:], gsc[m][:, e:e + 1])
                        else:
                            nc.vector.scalar_tensor_tensor(
                                osl, ps[:], gsc[m][:, e:e + 1], osl,
                                op0=OP.mult, op1=OP.add)
                    if e == 1 and m == 3:
                        # block-mask count: one ones^T @ mask matmul
                        cnt_ps = ps_lg.tile([1, MT * E], f32, tag="lg",
                                            bufs=1)
                        nc.tensor.matmul(cnt_ps[:], ones_col[:], mask_all[:],
                                         start=True, stop=True)
                        cnt_sb = p_gs.tile([1, MT * E], f32, tag="cnt_sb")
                        nc.vector.tensor_copy(cnt_sb[:], cnt_ps[:])
                        cnt_e = p_gs.tile([1, E], f32, tag="cnt_e")
                        nc.vector.tensor_reduce(
                            cnt_e[:],
                            cnt_sb[:].rearrange("p (m e) -> p e m", e=E),
                            axis=AX.X, op=OP.add,
                        )
                        bm01 = p_gs.tile([1, E], bf16, tag="bm01")
                        nc.vector.tensor_scalar(bm01[:], cnt_e[:], 0.5, None,
                                                op0=OP.is_ge)
                        # fold the fp8 dequant scale into the mask row
                        nc.vector.tensor_tensor(bm01[:], bm01[:], dq_row[:],
                                                op=OP.mult)
                    if e == 1 and m == 5:
                        # broadcast [1,E] -> [P,E] via K=1 matmul
                        bmb_ps = ps_lg.tile([P, E], f32, tag="lg", bufs=1)
                        nc.tensor.matmul(bmb_ps[:], ones_row_bf[:], bm01[:],
                                         start=True, stop=True)
                        nc.vector.tensor_copy(bmb[:], bmb_ps[:])
                        for mm in range(MT):
                            nc.vector.tensor_tensor(gsc[mm][:], gfin[mm][:],
                                                    bmb[:], op=OP.mult)
                    if e in (3, 4):
                        # merge the unscaled experts: acc += g0*acc0 (e 3)
                        # and acc += g1*acc1 (e 4), all m tiles. Keeping
                        # these DVE riders OFF the fp8 experts matters: a
                        # DoubleRow iteration is ~1.8us of PE vs ~2us of
                        # DVE with a merge attached, and the PSUM pool
                        # stalls the PE behind the drain (measured ~14ns/MM
                        # across experts 5-6 when they carried merges).
                        merge_e = e - 3
                        a_un = acc0 if merge_e == 0 else acc1
                        gcol = gsc[m][:, merge_e:merge_e + 1]
                        for h in range(2):
                            osl = acc[m][:, h * NH:(h + 1) * NH]
                            asl = a_un[m][:, h * NH:(h + 1) * NH]
                            nc.vector.scalar_tensor_tensor(
                                osl, asl, gcol, osl,
                                op0=OP.mult, op1=OP.add)

            # -- expert 7, half-major: all m-tiles of columns 0..511 first,
            # then 512..1023. Each (m, half)'s combine writes the bf16
            # output tile and its DMA overlaps the remaining matmuls, so
            # the post-stream tail is one half-tile deep.
            wbf = wbf_cur
            for h in range(2):
                for m in range(MT):
                    ps = ps_mm.tile([P, NH], f32, tag="psmm")
                    for k in range(KT):
                        lhs = xtb[k][:, m * P:(m + 1) * P]
                        nc.tensor.matmul(ps[:], lhs, wbf[k][:, h * NH:
                                                            (h + 1) * NH],
                                         start=(k == 0), stop=(k == KT - 1))
                    osl = acc[m][:, h * NH:(h + 1) * NH]
                    obl = outb[m][:, h * NH:(h + 1) * NH]
                    # fused (ps*g7)+acc -> bf16 output tile in ONE DVE op.
                    nc.vector.scalar_tensor_tensor(
                        obl, ps[:], gsc[m][:, E - 1:E], osl,
                        op0=OP.mult, op1=OP.add)
                    if h == 1:
                        # one full-row DMA per m (h=0's half is already in
                        # outb): 8 DMAs instead of 16 - each DMA costs an
                        # event in the serial end-of-kernel sweep (~0.1us
                        # per event per engine).
                        nc.sync.dma_start(out_r[m], outb[m][:])

    nc.compile()
    return nc


def _ensure_ntff_hook_module():
    """Defensive: some environments lack ``antenv.axon_hooks``; if a caller
    sets BASS_TRACE=1, run_bass_kernel_spmd imports it unconditionally and
    would crash. Provide a working shim (wired to the axon profiler if the
    library is present, else a no-hook stub)."""
    import sys
    import types

    try:
        import antenv.axon_hooks  # noqa: F401
        return
    except ImportError:
        pass
    try:
        import antenv  # noqa: F401
    except ImportError:
        return
    m = types.ModuleType("antenv.axon_hooks")
    exec(
        "_hook = None\n"
        "def set_axon_ntff_profile_hook(h):\n"
        "    global _hook\n"
        "    _hook = h\n"
        "def get_axon_ntff_profile_hook():\n"
        "    return _hook\n",
        m.__dict__,
    )
    sys.modules["antenv.axon_hooks"] = m
    try:
        from trn_agent_boot.trn_boot import _ntff_profile_via_ctypes

        m.set_axon_ntff_profile_hook(
            _ntff_profile_via_ctypes("/opt/axon/libaxon_pjrt.so")
        )
    except Exception:
        pass


_ensure_ntff_hook_module()

_CACHE = {}
LAST_RESULTS = None  # BassKernelResults of the most recent run (for test.py)


def _get_nc():
    if "nc" not in _CACHE:
        _CACHE["nc"] = _build_nc()
    return _CACHE["nc"]


def kernel(x, W, gate_w, gate_b):
    global LAST_RESULTS
    import ml_dtypes
    from concourse.bass_utils import run_bass_kernel_spmd

    bf = ml_dtypes.bfloat16
    e4 = ml_dtypes.float8_e4m3
    x = np.asarray(x, dtype=np.float32)
    W32 = np.asarray(W, dtype=np.float32)
    W_bf = np.ascontiguousarray(W32.astype(bf))
    w8_np = np.ascontiguousarray(np.concatenate(
        [np.clip(W32[:, e * D:(e + 1) * D] * SWQ, -240, 240)
         for e in FP8_EXPERTS], axis=1).astype(e4))
    gw_bf = np.ascontiguousarray(
        np.asarray(gate_w, dtype=np.float32).astype(bf))
    gb_bf = np.ascontiguousarray(np.tile(
        np.asarray(gate_b, dtype=np.float32).astype(bf).reshape(1, E),
        (1, MT)))

    W0 = W_bf[:, 0:D]
    in_maps = []
    for c in range(N_CORES):
        xT32 = x[c * TOK:(c + 1) * TOK].T
        xT = xT32.astype(bf)
        x8T = np.ascontiguousarray(
            np.clip(xT32 * SXQ, -240, 240).astype(e4))
        xw0 = np.ascontiguousarray(np.concatenate([xT, W0], axis=1))
        in_maps.append(
            {"xw0": xw0, "w": W_bf, "x8": x8T, "w8": w8_np,
             "gate_w": gw_bf, "gate_b": gb_bf})

    res = run_bass_kernel_spmd(_get_nc(), in_maps, core_ids=list(range(N_CORES)))
    LAST_RESULTS = res
    return np.concatenate(
        [r["out"].astype(np.float32) for r in res.results], axis=0)



# revision 44
# speedup vs baseline: 1.0121x; 1.0121x over previous
"""Trainium2 Bass kernel for nn_MoELayer (moe_routing).

Reference computation (B=8192 tokens, d=1024, E=8 experts, top-k=2):
    gating  = softmax(x @ gate_w + gate_b)                    # [B, E]
    mask    = top-2 one-hot scatter of gating                 # [B, E]
    blockm  = mask.reshape(B//d, d, E).max(axis=1)            # per 1024-row block
    out     = sum_e gating[:, e] * blockm[block(b), e] * (x @ W[:, e*d:(e+1)*d])

Key structural facts exploited here:
  * The combine uses the FULL softmax weights over all experts; the top-2
    mask enters only through the per-1024-row-block max. So the compute is
    dense: out = sum_e (gating*blockmask) .* (x @ W_e).
  * Both the gating and the block mask for a 1024-row block depend only on
    that block's rows.

Sharding: data-parallel over the 8 row blocks of 1024 tokens (one per
NeuronCore). Each core computes its whole output block locally; there is
no cross-core communication. W is streamed (replicated) to every core.

Precision: experts 5 and 6 are computed in fp8 e4m3 with
perf_mode=DoubleRow (two k-tiles contracted per matmul -> ~2x the bf16
MAC rate; a K=256,N=512 DoubleRow matmul measures ~same ns as a K=128
bf16 one). Host quantizes x*4 and W_e*128 to e4m3; the 1/512 dequant is
folded into the per-expert block-mask row, so the combine path is
unchanged. Measured end-to-end rel err 1.89e-2 vs the 2e-2 gate
(bf16-only is 0.33e-2; each fp8 expert adds ~1.33e-2 in quadrature -
exactly two experts fit the budget). This cut HW time 241.3us -> 214.2us.

Schedule (PE-bound: 6 bf16 experts at the 256B/cycle moving-operand
streaming bound + 2 fp8 DoubleRow experts at 2x):
  * All inputs are host-converted (bf16, or e4m3 for the fp8 experts):
    no on-device casts gate the matmul stream.
  * ALL PE-consumed inputs ride the ONE Sync HW queue as per-k-tile 2D
    DMAs ([xT | W0] pairs first): cross-queue operand waits throttle the
    in-order PE queue, 3D access patterns blow up descriptor generation
    ~14x, and the scalar queue doesn't move user data until ~13us (all
    three were measured and reverted).
  * Expert 0 is computed UNSCALED in two k-half passes (starts as soon
    as the first k-tiles arrive, no gating dependency); expert 1 likewise.
    Their gating scales are folded in during experts 3-6.
  * Gating logits ride the expert-0 stream as N=8 matmuls reusing the
    already-loaded x-tile stationary (k-accumulated into one PSUM bank),
    + K=1 bias matmuls; softmax/top-2/block-mask run under expert 1.
  * Experts 2..6: bf16 matmuls, fp32 PSUM k-accumulation; ACT applies the
    per-token gating scale PSUM->SBUF; DVE accumulates into acc.
  * Expert 7 runs half-major (all m-tiles of columns 0-511, then 512-1023)
    writing bf16 output tiles; each half's output DMA overlaps the rest.
  * Output is bf16 (host upcasts to fp32): halves the output DMA.
  * A short PE warm-up keeps the HAM clock-gate at 2.4 GHz from the start.
"""

import numpy as np

P = 128          # partitions
D = 1024         # d_model
E = 8            # experts
TOK = 1024       # tokens per core (row block)
KT = D // P      # contraction tiles
KH = KT // 2     # k-half for expert 0's two passes
MT = TOK // P    # token tiles
NH = 512         # psum half-width (one fp32 bank)
N_CORES = 8
WARMUP_MMS = 22  # N=128 PE warm-up matmuls issued while the first DMAs land
                 # (22 is tuned: pass A is DMA-arrival-paced, and extra
                 # warm-up MMs sit in FRONT of it on the in-order PE queue -
                 # 36 was measured +1.5us end-to-end)

# Experts computed in fp8 e4m3 with DoubleRow (2 fp8 weights/PE cell -> a
# K=256 matmul per instruction, ~1.8x the bf16 MAC rate). The e4m3
# quantization noise of one full expert adds ~1.3% rel error (measured
# end-to-end: bf16-only 0.33%, per-expert fp8 variance 4*0.44%^2); the
# 2e-2 gate leaves room for 2 experts (~1.9%). The dequant scale
# 1/(SXQ*SWQ) rides the per-expert block-mask row bm01, so the combine
# path is unchanged. Experts 0 (xw-packed) and 7 (output tail) stay bf16.
FP8_EXPERTS = (5, 6)
KP = KT // 2     # DoubleRow k-tile pairs per expert
SXQ = 4.0        # x -> e4m3 scale (|4x| <= ~22 << 240)
SWQ = 128.0      # W -> e4m3 scale (lifts W*0.01 out of the subnormal range)
NF8 = len(FP8_EXPERTS)


def _build_nc():
    import concourse.bacc as bacc
    import concourse.mybir as mybir
    import concourse.tile as tile

    f32 = mybir.dt.float32
    bf16 = mybir.dt.bfloat16
    f8 = mybir.dt.float8e4
    PM = mybir.MatmulPerfMode.DoubleRow
    AX = mybir.AxisListType
    OP = mybir.AluOpType
    AF = mybir.ActivationFunctionType

    # Bacc (not raw Bass): its compile() pass splits excess per-instruction
    # semaphore waits into standalone event-semaphore instructions and moves
    # matmul waits onto LDWEIGHTS — required for walrus codegen's per-
    # instruction sync-wait limits.
    assert 0 not in FP8_EXPERTS and (E - 1) not in FP8_EXPERTS
    nc = bacc.Bacc(None, target_bir_lowering=False, debug=False)
    # xw0: host-packed [xT | W_expert0] row blocks - one DMA per k-tile
    # delivers BOTH of the first matmuls' operands with one issue slot and
    # one completion event (the startup trickle is issue-rate + BW bound).
    xw_d = nc.dram_tensor("xw0", [D, 2 * D], bf16, kind="ExternalInput")
    w_d = nc.dram_tensor("w", [D, E * D], bf16, kind="ExternalInput")
    x8_d = nc.dram_tensor("x8", [D, TOK], f8, kind="ExternalInput")
    w8_d = nc.dram_tensor("w8", [D, NF8 * D], f8, kind="ExternalInput")
    gw_d = nc.dram_tensor("gate_w", [D, E], bf16, kind="ExternalInput")
    gb_d = nc.dram_tensor("gate_b", [1, MT * E], bf16, kind="ExternalInput")
    out_d = nc.dram_tensor("out", [TOK, D], bf16, kind="ExternalOutput")

    xw_r = xw_d.rearrange("(k p) f -> k p f", p=P)
    w_r2 = w_d.rearrange("(k p) (e f) -> p e k f", p=P, f=D)
    x8_r = x8_d.rearrange("(k p) t -> p k t", p=P)
    w8_r = w8_d.rearrange("(k p) (i f) -> p i k f", p=P, f=D)
    gw_r = gw_d.rearrange("(k p) e -> p k e", p=P)
    out_r = out_d.rearrange("(m p) f -> m p f", p=P)

    with tile.TileContext(nc) as tc:
        with (
            tc.tile_pool(name="persist", bufs=1) as persist,
            tc.tile_pool(name="gstat", bufs=2) as p_gs,
            tc.tile_pool(name="wb", bufs=KT) as p_wb,
            tc.tile_pool(name="wbig", bufs=2) as p_wbig,
            tc.tile_pool(name="w8p", bufs=2) as p_w8,
            tc.tile_pool(name="ps_lg", bufs=1, space="PSUM") as ps_lg,
            tc.tile_pool(name="ps_mm", bufs=7, space="PSUM") as ps_mm,
        ):
            # -- input DMAs first: nothing depends on compute, so they are
            # issued before any front-matter. ALL PE-consumed inputs ride
            # the ONE sync HW queue: a matmul whose lhsT and rhs complete on
            # different DMA queues needs two DMA-queue waits, which the
            # LDWEIGHTS wait encoding can't take. x/w0 k-tiles interleaved
            # so the first matmul's operands land earliest; gate tensors
            # follow (their matmuls run in pass B, ~15us later).
            # ALL PE-consumed inputs ride the ONE sync HW queue: splitting
            # x onto the scalar queue was measured 12us SLOWER - every
            # expert matmul then needs waits on two DMA queues, which Bacc
            # legalizes as standalone event-semaphore instructions that
            # throttle the in-order PE queue for the whole stream.
            xtb = []
            wbf0 = []
            for k in range(KT):
                xw = p_wb.tile([P, 2 * D], bf16, tag="xw", name=f"xw{k}")
                nc.sync.dma_start(xw[:], xw_r[k])
                xtb.append(xw[:, 0:D])
                wbf0.append(xw[:, D:2 * D])
            gw_in = persist.tile([P, KT, E], bf16, tag="gw_in")
            nc.sync.dma_start(gw_in[:], gw_r[:])
            # gate_b host-tiled to [1, MT*E] so ONE K=1 matmul adds the
            # bias to every m-tile's logit region at once.
            gb_in = persist.tile([1, MT * E], bf16, tag="gb_in")
            nc.sync.dma_start(gb_in[:], gb_d[:])

            # -- front matter (no DMA dependency).
            # warm-up operand memsets on GpSimd: its preamble ends ~2us
            # before the Vector engine's, so the PE warm-up (and the HAM
            # clock ramp) starts pre-window instead of at ~7us. (A DMA-fed
            # variant was measured WORSE: the scalar HW queue doesn't move
            # user data until ~13us, stalling the whole in-order PE queue.)
            wu_t = persist.tile([P, 2 * P], bf16, tag="wu_t")
            nc.gpsimd.memset(wu_t[:], 0.0)
            wu_lhs = wu_t[:, 0:P]
            wu_rhs = wu_t[:, P:2 * P]
            ones_col = persist.tile([P, 1], bf16, tag="ones_col")
            nc.vector.memset(ones_col[:], 1.0)
            exp_in = persist.tile([P, 1], f32, tag="exp_in")
            nc.vector.memset(exp_in[:], 1.0)
            ones_row_bf = persist.tile([1, P], bf16, tag="ones_row_bf")
            nc.vector.memset(ones_row_bf[:], 1.0)
            # per-expert dequant row: 1.0, except 1/(SXQ*SWQ) for fp8
            # experts (folded into the block-mask row bm01 below).
            dq_row = persist.tile([1, E], bf16, tag="dq_row")
            nc.vector.memset(dq_row[:], 1.0)
            for _e in FP8_EXPERTS:
                nc.vector.memset(dq_row[:, _e:_e + 1], 1.0 / (SXQ * SWQ))
            # Preload the exp activation-table set (~2.7us) during DMA wait.
            exp_dummy = persist.tile([1, 1], f32, tag="exp_dummy")
            nc.scalar.activation(exp_dummy[:], exp_in[:1, :], AF.Exp)

            # PE warm-up while the first input DMAs are in flight: a dense
            # block of short N=128 matmuls keeps the HAM activity monitor
            # busy (and the clock ramping) until the real stream's operands
            # land, without ever delaying a ready matmul by more than
            # ~110ns. Warm-up, pass-A fillers and pass-B gating logits are
            # strictly sequential, so they time-share ONE PSUM bank (the
            # ps_lg pool) and the matmul pool gets 7 banks - a 6-bank pool
            # wraps every 3rd (m,e) iteration and was measured stalling the
            # PE ~216ns every 10.8us waiting on the DVE drain.
            wu_ps = ps_lg.tile([P, P], f32, tag="lg", bufs=1)
            for i in range(WARMUP_MMS):
                nc.tensor.matmul(
                    wu_ps[:], wu_lhs, wu_rhs,
                    start=(i == 0), stop=(i == WARMUP_MMS - 1),
                )

            bmb = persist.tile([P, E], f32, tag="bmb")

            # acc: the fp32 output accumulator. acc0/acc1: experts 0/1
            # computed UNSCALED (bf16; same precision as the bf16 matmul
            # inputs already cost). Their gating scales are folded in
            # during experts 3-6, off the startup critical path.
            acc = []
            acc0 = []
            acc1 = []
            outb = []
            for m in range(MT):
                acc.append(persist.tile([P, D], f32, tag=f"acc{m}",
                                        name=f"acc{m}"))
                acc0.append(persist.tile([P, D], bf16, tag=f"acc0{m}",
                                         name=f"acc0{m}"))
                acc1.append(persist.tile([P, D], bf16, tag=f"acc1{m}",
                                         name=f"acc1{m}"))
                outb.append(persist.tile([P, D], bf16, tag=f"outb{m}",
                                         name=f"outb{m}"))

            # Dependency-free filler matmul: keeps the PE's HAM activity
            # monitor busy during arrival-gated stretches so the clock stays
            # at 2.4 GHz.
            def pe_filler(n=1):
                for _ in range(n):
                    nc.tensor.matmul(wu_ps[:], wu_lhs, wu_rhs,
                                     start=True, stop=True)

            # -- expert 0, pass A (k = 0..3), unscaled -> acc0.
            for m in range(MT):
                ps0 = ps_mm.tile([P, NH], f32, tag="psmm")
                ps1 = ps_mm.tile([P, NH], f32, tag="psmm")
                for k in range(KH):
                    lhs = xtb[k][:, m * P:(m + 1) * P]
                    nc.tensor.matmul(ps0[:], lhs, wbf0[k][:, 0:NH],
                                     start=(k == 0), stop=(k == KH - 1))
                    nc.tensor.matmul(ps1[:], lhs, wbf0[k][:, NH:D],
                                     start=(k == 0), stop=(k == KH - 1))
                    if m == 0 and k > 0:
                        # 8 fillers + 2 cold wides ~= the 1.64us packed-pair
                        # DMA arrival cadence, so m0's k-loop never idles
                        # (measured ~1us of recurring idle at 4 fillers).
                        # k==0 needs none: the warm-up block still runs when
                        # pair 0 lands.
                        pe_filler(8)
                nc.scalar.copy(acc0[m][:, 0:NH], ps0[:])
                nc.scalar.copy(acc0[m][:, NH:D], ps1[:])

            def load_w(e):
                # one dma_start for the whole expert ([P, KT, D] tile):
                # fewer sync-queue instructions and end-of-kernel events.
                wb = p_wbig.tile([P, KT, D], bf16, tag="wbig", name=f"wb{e}")
                nc.sync.dma_start(wb[:], w_r2[:, e])
                return [wb[:, k, :] for k in range(KT)]

            def load_w8(e):
                # fp8 expert weights: [P, k-tile, D] e4m3, half the bytes.
                i = FP8_EXPERTS.index(e)
                wb = p_w8.tile([P, KT, D], f8, tag="w8", name=f"w8_{e}")
                nc.sync.dma_start(wb[:], w8_r[:, i])
                return wb

            # Prefetch expert 1's weights ahead of pass B.
            wbf_cur = load_w(1)
            # fp8 x^T (stationary side of the DoubleRow matmuls): issued
            # after W1 so it cannot delay pass B; first consumed by the
            # earliest fp8 expert, many tens of us later.
            x8t = persist.tile([P, KT, TOK], f8, tag="x8t")
            nc.sync.dma_start(x8t[:], x8_r[:])

            # Gating logits accumulate in one PSUM bank: all 8 m-tiles side
            # by side ([P, MT*E]), each m-tile k-accumulated by the tiny
            # matmul chains riding pass B below. Allocated here (after the
            # last pe_filler) because the bank is time-shared with the
            # warm-up/filler tile.
            lg_all = ps_lg.tile([P, MT * E], f32, tag="lg", bufs=1)

            # -- expert 0, pass B (k = 4..7), accumulate into acc0 on DVE.
            # Each m's gating logits ride along as a contiguous chain of
            # N=8 matmuls over all k (reusing resident x-tiles) + the K=1
            # bias matmul. One chain per m, start->stop contiguous in the
            # lg bank (start=True's whole-bank has_written clear only
            # resets bits of finished chains, whose values just get read).
            for m in range(MT):
                ps0 = ps_mm.tile([P, NH], f32, tag="psmm")
                ps1 = ps_mm.tile([P, NH], f32, tag="psmm")
                for k in range(KH, KT):
                    lhs = xtb[k][:, m * P:(m + 1) * P]
                    nc.tensor.matmul(ps0[:], lhs, wbf0[k][:, 0:NH],
                                     start=(k == KH), stop=(k == KT - 1))
                    nc.tensor.matmul(ps1[:], lhs, wbf0[k][:, NH:D],
                                     start=(k == KH), stop=(k == KT - 1))
                for k in range(KT):
                    # single start=True on the very first gating matmul: it
                    # clears has_written for the whole bank; every other
                    # region's first write overwrites-then-sets, later k's
                    # accumulate.
                    nc.tensor.matmul(lg_all[:, m * E:(m + 1) * E],
                                     xtb[k][:, m * P:(m + 1) * P],
                                     gw_in[:, k, :],
                                     start=(m == 0 and k == 0), stop=False)
                nc.vector.tensor_tensor(acc0[m][:, 0:NH], acc0[m][:, 0:NH],
                                        ps0[:], op=OP.add)
                nc.vector.tensor_tensor(acc0[m][:, NH:D], acc0[m][:, NH:D],
                                        ps1[:], op=OP.add)
            # one K=1 matmul adds the (host-tiled) bias to ALL m-regions
            # and closes the whole logit accumulation.
            nc.tensor.matmul(lg_all[:], ones_row_bf[:], gb_in[:],
                             start=False, stop=True)

            # Gating part 2: ONE ACT Exp over the whole [P, MT*E] logit bank
            # (a single PSUM read, so the PE's later bank writes never
            # ping-pong with per-m reads), then the softmax / top-2-mask
            # chain on SBUF slices. The cross-m reductions collapse to one
            # strided tensor_reduce each.
            ex_all = persist.tile([P, MT * E], f32, tag="ex_all")
            nc.scalar.activation(ex_all[:], lg_all[:], AF.Exp)
            ssum_all = p_gs.tile([P, MT], f32, tag="ssum_all", bufs=1)
            nc.vector.tensor_reduce(
                ssum_all[:], ex_all[:].rearrange("p (m e) -> p m e", e=E),
                axis=AX.X, op=OP.add,
            )
            rcp_all = p_gs.tile([P, MT], f32, tag="rcp_all", bufs=1)
            nc.vector.reciprocal(rcp_all[:], ssum_all[:])
            m1_all = p_gs.tile([P, MT], f32, tag="m1_all", bufs=1)
            nc.vector.tensor_reduce(
                m1_all[:], ex_all[:].rearrange("p (m e) -> p m e", e=E),
                axis=AX.X, op=OP.max,
            )
            mask_all = persist.tile([P, MT * E], bf16, tag="mask_all")
            gfin = []
            gsc = [persist.tile([P, E], f32, tag=f"gsc{m}", name=f"gsc{m}")
                   for m in range(MT)]
            for m in range(MT):
                ex = ex_all[:, m * E:(m + 1) * E]
                # top-2 mask: v >= (max of values with the argmax removed)
                eqb = p_gs.tile([P, E], f32, tag="eqb")
                nc.vector.tensor_scalar(
                    eqb[:], ex, m1_all[:, m:m + 1], -1e30,
                    op0=OP.is_ge, op1=OP.mult
                )
                g2 = p_gs.tile([P, E], f32, tag="g2")
                nc.vector.tensor_tensor(g2[:], ex, eqb[:], op=OP.add)
                m2 = p_gs.tile([P, 1], f32, tag="m2")
                nc.vector.reduce_max(m2[:], g2[:], axis=AX.X)
                nc.vector.tensor_scalar(mask_all[:, m * E:(m + 1) * E],
                                        ex, m2[:], None, op0=OP.is_ge)
                gt = p_gs.tile([P, E], f32, tag=f"gt{m}", bufs=1)
                nc.vector.tensor_scalar_mul(gt[:], ex, rcp_all[:, m:m + 1])
                gfin.append(gt)

            # -- experts 1..6: acc (+)= g_e * (x @ W_e) via ONE fused DVE
            # op per half (scalar_tensor_tensor: (ps*g)+acc). Expert 0/1's
            # scaled contributions are merged in during experts 3-6. The
            # block-mask matmuls ride the PE queue inside expert 1's m-loop
            # (late enough that the softmax chain has finished - placing
            # them right after pass B would stall the in-order PE queue).
            for e in range(1, E - 1):
                wbf = wbf_cur
                cur_f8 = e in FP8_EXPERTS
                nxt = e + 1
                wbf_cur = load_w8(nxt) if nxt in FP8_EXPERTS else load_w(nxt)
                for m in range(MT):
                    ps0 = ps_mm.tile([P, NH], f32, tag="psmm")
                    ps1 = ps_mm.tile([P, NH], f32, tag="psmm")
                    if cur_f8:
                        # DoubleRow: each matmul contracts a PAIR of
                        # k-tiles ([128, 2, *] APs) at ~2x the MAC rate.
                        for j in range(KP):
                            lhs = x8t[:, 2 * j:2 * j + 2, m * P:(m + 1) * P]
                            nc.tensor.matmul(ps0[:], lhs,
                                             wbf[:, 2 * j:2 * j + 2, 0:NH],
                                             start=(j == 0),
                                             stop=(j == KP - 1), perf_mode=PM)
                            nc.tensor.matmul(ps1[:], lhs,
                                             wbf[:, 2 * j:2 * j + 2, NH:D],
                                             start=(j == 0),
                                             stop=(j == KP - 1), perf_mode=PM)
                    else:
                        for k in range(KT):
                            lhs = xtb[k][:, m * P:(m + 1) * P]
                            nc.tensor.matmul(ps0[:], lhs, wbf[k][:, 0:NH],
                                             start=(k == 0),
                                             stop=(k == KT - 1))
                            nc.tensor.matmul(ps1[:], lhs, wbf[k][:, NH:D],
                                             start=(k == 0),
                                             stop=(k == KT - 1))
                    for h, ps in ((0, ps0), (1, ps1)):
                        osl = acc[m][:, h * NH:(h + 1) * NH]
                        if e == 1:
                            # expert 1 is also computed unscaled (no gating
                            # dependency); merged with its gate later.
                            nc.scalar.copy(acc1[m][:, h * NH:(h + 1) * NH],
                                           ps[:])
                        elif e == 2:
                            nc.scalar.mul(osl, ps[:], gsc[m][:, e:e + 1])
                        else:
                            nc.vector.scalar_tensor_tensor(
                                osl, ps[:], gsc[m][:, e:e + 1], osl,
                                op0=OP.mult, op1=OP.add)
                    if e == 1 and m == 3:
                        # block-mask count: one ones^T @ mask matmul
                        cnt_ps = ps_lg.tile([1, MT * E], f32, tag="lg",
                                            bufs=1)
                        nc.tensor.matmul(cnt_ps[:], ones_col[:], mask_all[:],
                                         start=True, stop=True)
                        cnt_sb = p_gs.tile([1, MT * E], f32, tag="cnt_sb")
                        nc.vector.tensor_copy(cnt_sb[:], cnt_ps[:])
                        cnt_e = p_gs.tile([1, E], f32, tag="cnt_e")
                        nc.vector.tensor_reduce(
                            cnt_e[:],
                            cnt_sb[:].rearrange("p (m e) -> p e m", e=E),
                            axis=AX.X, op=OP.add,
                        )
                        bm01 = p_gs.tile([1, E], bf16, tag="bm01")
                        nc.vector.tensor_scalar(bm01[:], cnt_e[:], 0.5, None,
                                                op0=OP.is_ge)
                        # fold the fp8 dequant scale into the mask row
                        nc.vector.tensor_tensor(bm01[:], bm01[:], dq_row[:],
                                                op=OP.mult)
                    if e == 1 and m == 5:
                        # broadcast [1,E] -> [P,E] via K=1 matmul
                        bmb_ps = ps_lg.tile([P, E], f32, tag="lg", bufs=1)
                        nc.tensor.matmul(bmb_ps[:], ones_row_bf[:], bm01[:],
                                         start=True, stop=True)
                        nc.vector.tensor_copy(bmb[:], bmb_ps[:])
                        for mm in range(MT):
                            nc.vector.tensor_tensor(gsc[mm][:], gfin[mm][:],
                                                    bmb[:], op=OP.mult)
                    if e in (3, 4):
                        # merge the unscaled experts: acc += g0*acc0 (e 3)
                        # and acc += g1*acc1 (e 4), all m tiles. Keeping
                        # these DVE riders OFF the fp8 experts matters: a
                        # DoubleRow iteration is ~1.8us of PE vs ~2us of
                        # DVE with a merge attached, and the PSUM pool
                        # stalls the PE behind the drain (measured ~14ns/MM
                        # across experts 5-6 when they carried merges).
                        merge_e = e - 3
                        a_un = acc0 if merge_e == 0 else acc1
                        gcol = gsc[m][:, merge_e:merge_e + 1]
                        for h in range(2):
                            osl = acc[m][:, h * NH:(h + 1) * NH]
                            asl = a_un[m][:, h * NH:(h + 1) * NH]
                            nc.vector.scalar_tensor_tensor(
                                osl, asl, gcol, osl,
                                op0=OP.mult, op1=OP.add)

            # -- expert 7, half-major: all m-tiles of columns 0..511 first,
            # then 512..1023. Each (m, half)'s combine writes the bf16
            # output tile and its DMA overlaps the remaining matmuls, so
            # the post-stream tail is one half-tile deep.
            wbf = wbf_cur
            for h in range(2):
                for m in range(MT):
                    ps = ps_mm.tile([P, NH], f32, tag="psmm")
                    for k in range(KT):
                        lhs = xtb[k][:, m * P:(m + 1) * P]
                        nc.tensor.matmul(ps[:], lhs, wbf[k][:, h * NH:
                                                            (h + 1) * NH],
                                         start=(k == 0), stop=(k == KT - 1))
                    osl = acc[m][:, h * NH:(h + 1) * NH]
                    obl = outb[m][:, h * NH:(h + 1) * NH]
                    # fused (ps*g7)+acc -> bf16 output tile in ONE DVE op.
                    nc.vector.scalar_tensor_tensor(
                        obl, ps[:], gsc[m][:, E - 1:E], osl,
                        op0=OP.mult, op1=OP.add)
                    if h == 1:
                        # one full-row DMA per m (h=0's half is already in
                        # outb): 8 DMAs instead of 16 - each DMA costs an
                        # event in the serial end-of-kernel sweep (~0.1us
                        # per event per engine).
                        nc.sync.dma_start(out_r[m], outb[m][:])

    nc.compile()
    return nc


def _ensure_ntff_hook_module():
    """Defensive: some environments lack ``antenv.axon_hooks``; if a caller
    sets BASS_TRACE=1, run_bass_kernel_spmd imports it unconditionally and
    would crash. Provide a working shim (wired to the axon profiler if the
    library is present, else a no-hook stub)."""
    import sys
    import types

    try:
        import antenv.axon_hooks  # noqa: F401
        return
    except ImportError:
        pass
    try:
        import antenv  # noqa: F401
    except ImportError:
        return
    m = types.ModuleType("antenv.axon_hooks")
    exec(
        "_hook = None\n"
        "def set_axon_ntff_profile_hook(h):\n"
        "    global _hook\n"
        "    _hook = h\n"
        "def get_axon_ntff_profile_hook():\n"
        "    return _hook\n",
        m.__dict__,
    )
    sys.modules["antenv.axon_hooks"] = m
    try:
        from trn_agent_boot.trn_boot import _ntff_profile_via_ctypes

        m.set_axon_ntff_profile_hook(
            _ntff_profile_via_ctypes("/opt/axon/libaxon_pjrt.so")
        )
    except Exception:
        pass


_ensure_ntff_hook_module()

_CACHE = {}
LAST_RESULTS = None  # BassKernelResults of the most recent run (for test.py)


def _get_nc():
    if "nc" not in _CACHE:
        _CACHE["nc"] = _build_nc()
    return _CACHE["nc"]


def kernel(x, W, gate_w, gate_b):
    global LAST_RESULTS
    import ml_dtypes
    from concourse.bass_utils import run_bass_kernel_spmd

    bf = ml_dtypes.bfloat16
    e4 = ml_dtypes.float8_e4m3
    x = np.asarray(x, dtype=np.float32)
    W32 = np.asarray(W, dtype=np.float32)
    W_bf = np.ascontiguousarray(W32.astype(bf))
    w8_np = np.ascontiguousarray(np.concatenate(
        [np.clip(W32[:, e * D:(e + 1) * D] * SWQ, -240, 240)
         for e in FP8_EXPERTS], axis=1).astype(e4))
    gw_bf = np.ascontiguousarray(
        np.asarray(gate_w, dtype=np.float32).astype(bf))
    gb_bf = np.ascontiguousarray(np.tile(
        np.asarray(gate_b, dtype=np.float32).astype(bf).reshape(1, E),
        (1, MT)))

    W0 = W_bf[:, 0:D]
    in_maps = []
    for c in range(N_CORES):
        xT32 = x[c * TOK:(c + 1) * TOK].T
        xT = xT32.astype(bf)
        x8T = np.ascontiguousarray(
            np.clip(xT32 * SXQ, -240, 240).astype(e4))
        xw0 = np.ascontiguousarray(np.concatenate([xT, W0], axis=1))
        in_maps.append(
            {"xw0": xw0, "w": W_bf, "x8": x8T, "w8": w8_np,
             "gate_w": gw_bf, "gate_b": gb_bf})

    res = run_bass_kernel_spmd(_get_nc(), in_maps, core_ids=list(range(N_CORES)))
    LAST_RESULTS = res
    return np.concatenate(
        [r["out"].astype(np.float32) for r in res.results], axis=0)

